# revision 18
# baseline (speedup 1.0000x reference)
"""AttentionBlock kernel for 8 TRN2 NeuronCores — t-split + fp8 DoubleRow.

Reference (per batch b, T=2048, D=HID=1024):
    x = minibatch[b].T                      # [T, HID]
    m = x @ emb_w.T + emb_b                 # [T, D]
    K/Q/V = m @ W.T + b  (emb folded into combined weights on the host)
    logits = Q @ K.T  masked to t >= s else -32767
    probs = softmax(logits, axis=t) / 32    # softmax over the QUERY axis t
    read = probs @ V                        # contract over s
    out[b] = (read + m).T                   # [D, T]

Distribution: core c = 2*b + h owns batch b and the t-blocks {128*(2u+h)}
(interleaved for causal balance).  Each core computes Q and m only for its
own t-half, K and V for ALL s, logits/softmax/read for its own t columns.
The softmax normalization (over t!) needs cross-core stats: each core
computes per-key M_loc[s] = max_t logits and Z_loc[s] = sum_t exp(l - M_loc);
one tiny AllGather (32 KB) exchanges them, and
f[s] = exp(M_loc - M_glob) / (32 * Z_glob) is folded into V.  Each core's
read output is its own t-half of the final output — no reduce-scatter.

Precision: everything runs on fp8 DoubleRow matmuls.  The residual m uses a
three-term split-fp8 product (xh@Wh + xl@Wh + xh@Wl at a common 16x weight
scale) since plain fp8 there fails the 2e-2 gate.  Folded weights are scaled
16x on the host (their ~0.013 rms is fp8-subnormal) and rescaled in the
psum->sbuf copies.  E = exp(l - M_loc) and f*V are stored e5m2 for range
safety.  Measured end-to-end rel err vs the f32 reference: ~3e-3.

All per-core differences (t/s column permutation [own|peer], mask contents,
stat-merge blend weights) enter via input DATA — the graph is SPMD-identical.
"""

import sys

for _p in ("/opt/trn_rl_repo", "/opt/pypackages"):
    if _p not in sys.path:
        sys.path.insert(0, _p)

import numpy as np
import ml_dtypes

import concourse.bass as bass
import concourse.mybir as mybir
import concourse.tile as tile
from concourse import bacc
from concourse.bass_utils import run_bass_kernel_spmd

B, HID, T, D = 4, 1024, 2048, 1024
P = 128
SB = 16          # s-blocks of 128 (full T) per core
OT = 1024        # own t columns per core
NEGM = -60000.0  # additive mask value (acts as -inf through exp)
WS = 16.0        # host-side fp8 weight scale

BF = mybir.dt.bfloat16
F8 = mybir.dt.float8e4
E5 = mybir.dt.float8e5
F32 = mybir.dt.float32
DR = mybir.MatmulPerfMode.DoubleRow

PROFILE = False
LAST_EXEC_NS = None
_CACHE = {}


def _build_nc():
    nc = bacc.Bacc(None, target_bir_lowering=False, debug=False)

    x8 = nc.declare_dram_parameter("x8", [512, 2 * T], F8, isOutput=False)
    xl8 = nc.declare_dram_parameter("xl8", [512, 2 * OT], F8, isOutput=False)
    wq8 = nc.declare_dram_parameter("wq8", [512, 2 * D], F8, isOutput=False)
    wk8 = nc.declare_dram_parameter("wk8", [512, 2 * D], F8, isOutput=False)
    wv8 = nc.declare_dram_parameter("wv8", [512, 2 * D], F8, isOutput=False)
    mwh8 = nc.declare_dram_parameter("mwh8", [512, 2 * D], F8, isOutput=False)
    mwl8 = nc.declare_dram_parameter("mwl8", [512, 2 * D], F8, isOutput=False)
    masks = nc.declare_dram_parameter("masks", [SB * P, 512], BF, isOutput=False)
    ident = nc.declare_dram_parameter("ident", [P, P], BF, isOutput=False)
    bias4 = nc.declare_dram_parameter("bias4", [P, 26], F32, isOutput=False)
    vb = nc.declare_dram_parameter("vb", [1, D], BF, isOutput=False)
    onesv = nc.declare_dram_parameter("onesv", [1, P], BF, isOutput=False)
    out_ext = nc.declare_dram_parameter("out", [D, OT], BF, isOutput=True)

    stats_in = nc.dram_tensor("stats_in", [P, 32], F32)
    stats_out = nc.dram_tensor("stats_out", [2 * P, 32], F32)

    Ident = mybir.ActivationFunctionType.Identity
    Exp = mybir.ActivationFunctionType.Exp
    X = mybir.AxisListType.X
    MUL = mybir.AluOpType.mult
    ADD = mybir.AluOpType.add
    RG = [[0, 1], [2, 3], [4, 5], [6, 7]]

    with tile.TileContext(nc) as tc:
        with (
            tc.tile_pool(name="const", bufs=1) as const,
            tc.tile_pool(name="x8p", bufs=4) as x8p,
            tc.tile_pool(name="xlp", bufs=4) as xlp,
            tc.tile_pool(name="wp", bufs=12) as wp,
            tc.tile_pool(name="qp", bufs=4) as qp,
            tc.tile_pool(name="kp", bufs=4) as kp,
            tc.tile_pool(name="ep", bufs=8) as ep,
            tc.tile_pool(name="vp", bufs=8) as vp,
            tc.tile_pool(name="vcp", bufs=8) as vcp,
            tc.tile_pool(name="mp", bufs=8) as mp,
            tc.tile_pool(name="sxp", bufs=1) as sxp,
            tc.tile_pool(name="osp", bufs=6) as osp,
            tc.tile_pool(name="ps1", bufs=2, space="PSUM") as ps1,
            tc.tile_pool(name="ps5", bufs=4, space="PSUM") as ps5,
        ):
            # ---- x8 own-half + m inputs first (m runs before Q/K) ----
            x8t = []
            mwht = []
            mwlt = []
            xlt = []
            for j in range(4):
                t_ = x8p.tile([P, 2, T], F8, tag="x8", name=f"x8_{j}")
                nc.sync.dma_start(
                    t_[:, :, 0:OT],
                    x8[j * P : (j + 1) * P, :].rearrange("p (a t) -> p a t", a=2)[:, :, 0:OT],
                )
                x8t.append(t_)
                w_ = wp.tile([P, 2, D], F8, tag="w", name=f"mwh{j}")
                nc.sync.dma_start(
                    w_[:],
                    mwh8[j * P : (j + 1) * P, :].rearrange("p (a d) -> p a d", a=2),
                )
                mwht.append(w_)
                w2 = wp.tile([P, 2, D], F8, tag="w", name=f"mwl{j}")
                nc.sync.dma_start(
                    w2[:],
                    mwl8[j * P : (j + 1) * P, :].rearrange("p (a d) -> p a d", a=2),
                )
                mwlt.append(w2)
                t2 = xlp.tile([P, 2, OT], F8, tag="xl", name=f"xl{j}")
                nc.sync.dma_start(
                    t2[:], xl8[j * P : (j + 1) * P, :].rearrange("p (a t) -> p a t", a=2)
                )
                xlt.append(t2)

            # biases packed [128, 26]: eb 0:8, qb 8:16, kb 16:24, wtb 24:26
            b4 = const.tile([P, 26], F32)
            nc.sync.dma_start(b4[:], bias4[:])
            ebt, qbt, kbt, wtbt = b4[:, 0:8], b4[:, 8:16], b4[:, 16:24], b4[:, 24:26]

            # ---- m: split-fp8 (xh@Wh + xl@Wh + xh@Wl), own t cols ----
            mt = [mp.tile([P, OT], BF, tag="m", name=f"m{d}") for d in range(8)]
            for d in range(8):
                for i in range(2):
                    pt = ps5.tile([P, 512], F32, tag="p5", name=f"psm{d}_{i}")
                    nmm = 0
                    for j in range(4):
                        for lhs, rhs in (
                            (mwht[j], x8t[j][:, :, i * 512 : (i + 1) * 512]),
                            (mwht[j], xlt[j][:, :, i * 512 : (i + 1) * 512]),
                            (mwlt[j], x8t[j][:, :, i * 512 : (i + 1) * 512]),
                        ):
                            nmm += 1
                            nc.tensor.matmul(
                                pt[:], lhs[:, :, d * P : (d + 1) * P], rhs,
                                start=(nmm == 1), stop=(nmm == 12), perf_mode=DR,
                            )
                    nc.scalar.activation(
                        mt[d][:, i * 512 : (i + 1) * 512], pt[:], Ident,
                        bias=ebt[:, d : d + 1], scale=1.0 / WS,
                    )

            # ---- Q weights ----
            wqt = []
            for j in range(4):
                w_ = wp.tile([P, 2, D], F8, tag="w", name=f"wq{j}")
                nc.sync.dma_start(
                    w_[:],
                    wq8[j * P : (j + 1) * P, :].rearrange("p (a d) -> p a d", a=2),
                )
                wqt.append(w_)

            # ---- Q: own t cols (x8 cols 0:1024), paired over D for logits ----
            qt = [qp.tile([P, 2, OT], F8, tag="q", name=f"q{j}") for j in range(4)]
            for d in range(8):
                pt = ps1.tile([P, 1024], F32, tag="p1", name=f"psq{d}")
                for i in range(2):
                    for j in range(4):
                        nc.tensor.matmul(
                            pt[:, i * 512 : (i + 1) * 512],
                            wqt[j][:, :, d * P : (d + 1) * P],
                            x8t[j][:, :, i * 512 : (i + 1) * 512],
                            start=(j == 0), stop=(j == 3), perf_mode=DR,
                        )
                nc.scalar.activation(
                    qt[d // 2][:, d % 2, :], pt[:], Ident,
                    bias=qbt[:, d : d + 1], scale=1.0 / WS,
                )

            # peer half of x8 + K weights
            for j in range(4):
                nc.sync.dma_start(
                    x8t[j][:, :, OT:T],
                    x8[j * P : (j + 1) * P, :].rearrange("p (a t) -> p a t", a=2)[:, :, OT:T],
                )
            wkt = []
            for j in range(4):
                w_ = wp.tile([P, 2, D], F8, tag="w", name=f"wk{j}")
                nc.sync.dma_start(
                    w_[:],
                    wk8[j * P : (j + 1) * P, :].rearrange("p (a d) -> p a d", a=2),
                )
                wkt.append(w_)

            # ---- K: all 2048 s cols, paired over D ----
            kt = [kp.tile([P, 2, T], F8, tag="k", name=f"k{j}") for j in range(4)]
            for d in range(8):
                for q2 in range(2):
                    pt = ps1.tile([P, 1024], F32, tag="p1", name=f"psk{d}_{q2}")
                    for i in range(2):
                        for j in range(4):
                            nc.tensor.matmul(
                                pt[:, i * 512 : (i + 1) * 512],
                                wkt[j][:, :, d * P : (d + 1) * P],
                                x8t[j][:, :, q2 * 1024 + i * 512 : q2 * 1024 + (i + 1) * 512],
                                start=(j == 0), stop=(j == 3), perf_mode=DR,
                            )
                    dst = kt[d // 2][:, d % 2, q2 * 1024 : (q2 + 1) * 1024]
                    if q2 == 0:
                        nc.scalar.activation(
                            dst, pt[:], Ident, bias=kbt[:, d : d + 1], scale=1.0 / WS
                        )
                    else:
                        nc.vector.tensor_scalar(
                            dst, pt[:], 1.0 / WS, kbt[:, d : d + 1],
                            op0=MUL, op1=ADD,
                        )

            # remaining input DMAs (consumed in later phases)
            mskt = const.tile([P, SB, 512], BF)
            nc.sync.dma_start(mskt[:], masks.rearrange("(l p) c -> p l c", p=P))
            identt = const.tile([P, P], BF)
            nc.sync.dma_start(identt[:], ident[:])
            wvt = []
            for j in range(4):
                w_ = wp.tile([P, 2, D], F8, tag="w", name=f"wv{j}")
                nc.sync.dma_start(
                    w_[:],
                    wv8[j * P : (j + 1) * P, :].rearrange("p (a d) -> p a d", a=2),
                )
                wvt.append(w_)
            vbt = const.tile([1, D], BF)
            nc.sync.dma_start(vbt[:], vb[:])
            onest = const.tile([1, P], BF)
            nc.sync.dma_start(onest[:], onesv[:])

            # ---- logits + E + local stats, V halves interleaved ----
            mpack = sxp.tile([P, SB], F32)
            zpack = sxp.tile([P, SB], F32)
            et = [ep.tile([P, 2, OT], E5, tag="e", name=f"e{j}") for j in range(8)]
            vt = [vp.tile([P, 2, D], BF, tag="v", name=f"v{j}") for j in range(8)]

            def v_half(l, i, eng):
                pt = ps5.tile([P, 512], F32, tag="p5", name=f"psv{l}_{i}")
                for j in range(4):
                    nc.tensor.matmul(
                        pt[:],
                        x8t[j][:, :, l * P : (l + 1) * P],
                        wvt[j][:, :, i * 512 : (i + 1) * 512],
                        start=(j == 0), stop=False, perf_mode=DR,
                    )
                nc.tensor.matmul(
                    pt[:], onest[0:1, :], vbt[0:1, i * 512 : (i + 1) * 512],
                    start=False, stop=True,
                )
                dst = vt[l // 2][:, l % 2, i * 512 : (i + 1) * 512]
                if eng == 0:
                    nc.scalar.activation(dst, pt[:], Ident, scale=1.0 / WS)
                else:
                    nc.vector.tensor_scalar_mul(dst, pt[:], 1.0 / WS)

            # V block schedule: Vc pairs {0,1,4,5} = s-blocks 0-3 and 8-11 first
            VB1 = [0, 1, 2, 3, 8, 9, 10, 11]      # during logits l=0..7 (full)
            VB2 = [4, 5, 12, 13]                   # halves during l=8..15
            VB3 = [6, 7, 14, 15]                   # after the logits loop

            def logits_block(l):
                bnd = (l % 8) // 4      # boundary tile index == first computed
                pt = ps1.tile([P, 1024], F32, tag="p1", name=f"psl{l}")
                for i in range(bnd, 2):
                    for j in range(4):
                        nc.tensor.matmul(
                            pt[:, i * 512 : (i + 1) * 512],
                            kt[j][:, :, l * P : (l + 1) * P],
                            qt[j][:, :, i * 512 : (i + 1) * 512],
                            start=(j == 0), stop=(j == 3 and i != bnd),
                            perf_mode=DR,
                        )
                    if i == bnd:
                        nc.tensor.matmul(
                            pt[:, i * 512 : (i + 1) * 512],
                            identt[:], mskt[:, l, :],
                            start=False, stop=True,
                        )
                if l % 8 >= 4:
                    nc.vector.memset(et[l // 2][:, l % 2, 0:512], 0.0)
                sl = pt[:, bnd * 512 : 1024]
                mcol = mpack[:, l : l + 1]
                nc.vector.reduce_max(mcol, sl, axis=X)
                negM = sxp.tile([P, 1], F32, tag="ng", bufs=SB, name=f"ng{l}")
                nc.vector.tensor_scalar_mul(negM, mcol, -1.0)
                nc.scalar.activation(
                    et[l // 2][:, l % 2, bnd * 512 : 1024], sl, Exp,
                    bias=negM[:, 0:1], accum_out=zpack[:, l : l + 1],
                )

            for l in range(SB):
                logits_block(l)
                if l == 5:
                    v_half(0, 0, 1)
                    v_half(0, 1, 1)
                elif l == 10:
                    v_half(1, 0, 1)
                    v_half(1, 1, 1)

            # ---- stats exchange (one tiny AllGather per pair) ----
            spack = sxp.tile([P, 32], F32)
            nc.vector.tensor_copy(spack[:, 0:16], mpack[:])
            nc.vector.tensor_copy(spack[:, 16:32], zpack[:])
            nc.sync.dma_start(stats_in[:], spack[:])
            nc.gpsimd.collective_compute(
                "AllGather", mybir.AluOpType.bypass,
                ins=[stats_in[:]], outs=[stats_out[:]], replica_groups=RG,
            )
            gtop = sxp.tile([P, 32], F32)
            gbot = sxp.tile([P, 32], F32)
            nc.sync.dma_start(gtop[:], stats_out[0:P, :])
            nc.sync.dma_start(gbot[:], stats_out[P : 2 * P, :])

            # V bulk (blocks 0,1 done in-loop); Vc pairs {0,1,4,5} need 0-3, 8-11
            for n, vl in enumerate([2, 3, 8, 9, 10, 11, 4, 5, 6, 7, 12, 13, 14, 15]):
                v_half(vl, 0, n % 2)
                v_half(vl, 1, 1 - n % 2)

            # ---- combine stats -> f[s] = exp(M_loc - Mg) / (32 Zg) ----
            oth = sxp.tile([P, 32], F32)
            nc.vector.tensor_scalar_mul(oth[:], gtop[:], wtbt[:, 0:1])
            nc.vector.scalar_tensor_tensor(
                oth[:], gbot[:], wtbt[:, 1:2], oth[:], op0=MUL, op1=ADD
            )
            oM = sxp.tile([P, 16], F32)
            oZ = sxp.tile([P, 16], F32)
            nc.vector.tensor_copy(oM[:, 0:8], oth[:, 8:16])
            nc.vector.tensor_copy(oM[:, 8:16], oth[:, 0:8])
            nc.vector.tensor_copy(oZ[:, 0:8], oth[:, 24:32])
            nc.vector.tensor_copy(oZ[:, 8:16], oth[:, 16:24])
            mg = sxp.tile([P, 16], F32)
            nc.vector.tensor_max(mg[:], mpack[:], oM[:])
            dm = sxp.tile([P, 16], F32)
            nc.vector.tensor_sub(dm[:], mpack[:], mg[:])
            expm = sxp.tile([P, 16], F32)
            nc.scalar.activation(expm[:], dm[:], Exp)
            nc.vector.tensor_sub(dm[:], oM[:], mg[:])
            expo = sxp.tile([P, 16], F32)
            nc.scalar.activation(expo[:], dm[:], Exp)
            zg = sxp.tile([P, 16], F32)
            nc.vector.tensor_mul(zg[:], zpack[:], expm[:])
            nc.vector.tensor_mul(oZ[:], oZ[:], expo[:])
            nc.vector.tensor_add(zg[:], zg[:], oZ[:])
            fsc = sxp.tile([P, 16], F32)
            nc.vector.reciprocal(fsc[:], zg[:])
            nc.vector.tensor_mul(fsc[:], fsc[:], expm[:])
            nc.vector.tensor_scalar_mul(fsc[:], fsc[:], 1.0 / 32.0)

            # ---- Vc = f * V (e5m2); read tile i=0 only needs pairs {0,1,4,5} ----
            vct = [vcp.tile([P, 2, D], E5, tag="vc", name=f"vc{j}") for j in range(8)]

            def vc_pair(j2, eng):
                for a in range(2):
                    l = 2 * j2 + a
                    dst = vct[j2][:, a, :]
                    src = vt[j2][:, a, :]
                    if eng == 0:
                        nc.scalar.activation(dst, src, Ident, scale=fsc[:, l : l + 1])
                    elif eng == 1:
                        nc.vector.tensor_scalar_mul(dst, src, fsc[:, l : l + 1])
                    else:
                        nc.gpsimd.tensor_scalar_mul(dst, src, fsc[:, l : l + 1])

            def read_tile(i, prs):
                for d in range(8):
                    pt = ps5.tile([P, 512], F32, tag="p5", name=f"psr{i}_{d}")
                    for idx, j2 in enumerate(prs):
                        nc.tensor.matmul(
                            pt[:],
                            vct[j2][:, :, d * P : (d + 1) * P],
                            et[j2][:, :, i * 512 : (i + 1) * 512],
                            start=(idx == 0), stop=(idx == len(prs) - 1),
                            perf_mode=DR,
                        )
                    osb = osp.tile([P, 512], BF, tag="os", name=f"os{i}_{d}")
                    nc.vector.scalar_tensor_tensor(
                        osb[:], mt[d][:, i * 512 : (i + 1) * 512], 1.0, pt[:],
                        op0=MUL, op1=ADD,
                    )
                    nc.sync.dma_start(
                        out_ext[d * P : (d + 1) * P, i * 512 : (i + 1) * 512], osb[:]
                    )

            for n, j2 in enumerate((0, 1, 4, 5)):
                vc_pair(j2, n % 2)
            read_tile(0, [0, 1, 4, 5])
            for n, j2 in enumerate((2, 3, 6, 7)):
                vc_pair(j2, n % 2)
            read_tile(1, list(range(8)))

    nc.compile()
    return nc


def _prep_inputs(minibatch, emb_w, emb_b, key_w, key_b, query_w, query_b,
                 value_w, value_b):
    bf = ml_dtypes.bfloat16
    f8 = ml_dtypes.float8_e4m3
    ewT_f = np.ascontiguousarray(emb_w.T).astype(np.float32)
    W_eq = ewT_f @ query_w.T.astype(np.float32)
    W_ek = ewT_f @ key_w.T.astype(np.float32)
    W_ev = ewT_f @ value_w.T.astype(np.float32)
    b_eq = emb_b @ query_w.T + query_b
    b_ek = emb_b @ key_w.T + key_b
    b_ev = emb_b @ value_w.T + value_b

    def pack_w(W):
        # [HID, D] -> [512, 2D]: row 128j+p, col a*D+d  holds W[256j+128a+p, d]
        W4 = W.reshape(4, 2, P, D)
        return np.ascontiguousarray(
            W4.transpose(0, 2, 1, 3).reshape(512, 2 * D)
        ).astype(f8)

    def pack_bias(v):
        return np.ascontiguousarray(v.reshape(8, P).T).astype(np.float32)

    # 16x scale lifts the ~0.013-rms folded weights out of fp8-subnormal range
    mwh_f32 = (WS * ewT_f).astype(f8).astype(np.float32)
    shared = {
        "wq8": pack_w(WS * W_eq),
        "wk8": pack_w(WS * W_ek),
        "wv8": pack_w(WS * W_ev),
        "mwh8": pack_w(mwh_f32),
        "mwl8": pack_w(WS * ewT_f - mwh_f32),
        "vb": (WS * b_ev).astype(bf)[None, :],
        "onesv": np.ones((1, P), dtype=bf),
        "ident": np.eye(P, dtype=bf),
    }

    in_maps = []
    for c in range(8):
        b, h = c // 2, c % 2
        xbT = minibatch[b].astype(np.float32)          # [HID, T]
        own = np.concatenate(
            [np.arange(P * (2 * u + h), P * (2 * u + h) + P) for u in range(8)]
        )
        peer = np.concatenate(
            [np.arange(P * (2 * u + 1 - h), P * (2 * u + 1 - h) + P) for u in range(8)]
        )
        perm = np.concatenate([own, peer])             # col order [own | peer]
        xp = xbT[:, perm]                              # [HID, 2048] permuted

        def pack_x(xc, w):
            x4 = xc.reshape(4, 2, P, w)
            return np.ascontiguousarray(
                x4.transpose(0, 2, 1, 3).reshape(512, 2 * w)
            ).astype(f8)

        x8c = pack_x(xp, T)
        xh = xp[:, 0:OT].astype(f8).astype(np.float32)
        xl8c = pack_x(xp[:, 0:OT] - xh, OT)            # split-fp8 residual of x

        # masks: per s-block l (permuted order), boundary-tile content
        mk = np.zeros((SB * P, 512), dtype=np.float32)
        for l in range(SB):
            bnd = (l % 8) // 4
            base_pos = 4 * bnd
            if l < 8:
                phys = 2 * l + h                 # own-parity s block
            else:
                phys = 2 * (l - 8) + (1 - h)     # peer-parity s block
            srow = P * phys + np.arange(P)[:, None]
            for pos in range(base_pos, base_pos + 4):
                tcol = P * (2 * pos + h) + np.arange(P)[None, :]
                blk = (tcol < srow) * NEGM
                mk[l * P : (l + 1) * P, (pos - base_pos) * P : (pos - base_pos + 1) * P] = blk
        bias4 = np.zeros((P, 26), dtype=np.float32)
        bias4[:, 0:8] = pack_bias(emb_b.astype(np.float32))
        bias4[:, 8:16] = pack_bias(b_eq.astype(np.float32))
        bias4[:, 16:24] = pack_bias(b_ek.astype(np.float32))
        bias4[:, 24] = 1.0 if h == 1 else 0.0    # weight of gathered rank0 rows
        bias4[:, 25] = 1.0 if h == 0 else 0.0    # weight of gathered rank1 rows
        in_maps.append(dict(
            shared,
            x8=x8c,
            xl8=xl8c,
            masks=mk.astype(bf),
            bias4=bias4,
        ))
    return in_maps


def kernel(**inputs):
    global LAST_EXEC_NS
    inputs = {k: np.asarray(v) for k, v in inputs.items()}
    if "nc" not in _CACHE:
        _CACHE["nc"] = _build_nc()
    nc = _CACHE["nc"]
    in_maps = _prep_inputs(**inputs)
    kw = {}
    if PROFILE:
        kw["trace"] = True
    res = run_bass_kernel_spmd(nc, in_maps, core_ids=list(range(8)), **kw)
    LAST_EXEC_NS = getattr(res, "exec_time_ns", None)
    out = np.empty((B, D, T), dtype=np.float32)
    for c in range(8):
        b, h = c // 2, c % 2
        o = np.asarray(res.results[c]["out"]).astype(np.float32)  # [D, OT]
        own = np.concatenate(
            [np.arange(P * (2 * u + h), P * (2 * u + h) + P) for u in range(8)]
        )
        out[b][:, own] = o
    return out


# revision 19
# speedup vs baseline: 1.1920x; 1.1920x over previous
"""AttentionBlock kernel for 8 TRN2 NeuronCores — t-split + fp8 DoubleRow.

Reference (per batch b, T=2048, D=HID=1024):
    x = minibatch[b].T                      # [T, HID]
    m = x @ emb_w.T + emb_b                 # [T, D]
    K/Q/V = m @ W.T + b  (emb folded into combined weights on the host)
    logits = Q @ K.T  masked to t >= s else -32767
    probs = softmax(logits, axis=t) / 32    # softmax over the QUERY axis t
    read = probs @ V                        # contract over s
    out[b] = (read + m).T                   # [D, T]

Distribution: core c = 2*b + h owns batch b and the t-blocks {128*(2u+h)}
(interleaved for causal balance).  Each core computes Q and m only for its
own t-half, K and V for ALL s, logits/softmax/read for its own t columns.
The softmax normalization (over t!) needs cross-core stats: each core
computes per-key M_loc[s] = max_t logits and Z_loc[s] = sum_t exp(l - M_loc);
one tiny AllGather (32 KB) exchanges them, and
f[s] = exp(M_loc - M_glob) / (32 * Z_glob) is folded into V.  Each core's
read output is its own t-half of the final output — no reduce-scatter.

Precision: everything runs on fp8 DoubleRow matmuls.  The residual m uses a
three-term split-fp8 product (xh@Wh + xl@Wh + xh@Wl at a common 16x weight
scale) since plain fp8 there fails the 2e-2 gate.  Folded weights are scaled
16x on the host (their ~0.013 rms is fp8-subnormal) and rescaled in the
psum->sbuf copies.  E = exp(l - M_loc) and f*V are stored e5m2 for range
safety.  Measured end-to-end rel err vs the f32 reference: ~3e-3.

All per-core differences (t/s column permutation [own|peer], mask contents,
stat-merge blend weights) enter via input DATA — the graph is SPMD-identical.
"""

import sys

for _p in ("/opt/trn_rl_repo", "/opt/pypackages"):
    if _p not in sys.path:
        sys.path.insert(0, _p)

import numpy as np
import ml_dtypes

import concourse.bass as bass
import concourse.mybir as mybir
import concourse.tile as tile
from concourse import bacc
from concourse.bass_utils import run_bass_kernel_spmd

B, HID, T, D = 4, 1024, 2048, 1024
P = 128
SB = 16          # s-blocks of 128 (full T) per core
OT = 1024        # own t columns per core
NEGM = -60000.0  # additive mask value (acts as -inf through exp)
WS = 16.0        # host-side fp8 weight scale

BF = mybir.dt.bfloat16
F8 = mybir.dt.float8e4
E5 = mybir.dt.float8e5
F32 = mybir.dt.float32
DR = mybir.MatmulPerfMode.DoubleRow

PROFILE = False
LAST_EXEC_NS = None
_CACHE = {}


def _build_nc():
    nc = bacc.Bacc(None, target_bir_lowering=False, debug=False)

    x8 = nc.declare_dram_parameter("x8", [512, 2 * T], F8, isOutput=False)
    xl8 = nc.declare_dram_parameter("xl8", [512, 2 * OT], F8, isOutput=False)
    wq8 = nc.declare_dram_parameter("wq8", [512, 2 * D], F8, isOutput=False)
    wk8 = nc.declare_dram_parameter("wk8", [512, 2 * D], F8, isOutput=False)
    wv8 = nc.declare_dram_parameter("wv8", [512, 2 * D], F8, isOutput=False)
    mwh8 = nc.declare_dram_parameter("mwh8", [512, 2 * D], F8, isOutput=False)
    mwl8 = nc.declare_dram_parameter("mwl8", [512, 2 * D], F8, isOutput=False)
    masks = nc.declare_dram_parameter("masks", [SB * P, 512], BF, isOutput=False)
    ident = nc.declare_dram_parameter("ident", [P, P], BF, isOutput=False)
    bias4 = nc.declare_dram_parameter("bias4", [P, 26], F32, isOutput=False)
    vb = nc.declare_dram_parameter("vb", [1, D], BF, isOutput=False)
    onesv = nc.declare_dram_parameter("onesv", [1, P], BF, isOutput=False)
    out_ext = nc.declare_dram_parameter("out", [D, OT], BF, isOutput=True)

    stats_in = nc.dram_tensor("stats_in", [P, 32], F32)
    stats_out = nc.dram_tensor("stats_out", [2 * P, 32], F32)

    Ident = mybir.ActivationFunctionType.Identity
    Exp = mybir.ActivationFunctionType.Exp
    X = mybir.AxisListType.X
    MUL = mybir.AluOpType.mult
    ADD = mybir.AluOpType.add
    RG = [[0, 1], [2, 3], [4, 5], [6, 7]]

    with tile.TileContext(nc) as tc:
        with (
            tc.tile_pool(name="const", bufs=1) as const,
            tc.tile_pool(name="x8p", bufs=4) as x8p,
            tc.tile_pool(name="xlp", bufs=4) as xlp,
            tc.tile_pool(name="wp", bufs=12) as wp,
            tc.tile_pool(name="qp", bufs=4) as qp,
            tc.tile_pool(name="kp", bufs=4) as kp,
            tc.tile_pool(name="ep", bufs=8) as ep,
            tc.tile_pool(name="vp", bufs=8) as vp,
            tc.tile_pool(name="vcp", bufs=8) as vcp,
            tc.tile_pool(name="mp", bufs=8) as mp,
            tc.tile_pool(name="sxp", bufs=1) as sxp,
            tc.tile_pool(name="osp", bufs=6) as osp,
            tc.tile_pool(name="ps1", bufs=2, space="PSUM") as ps1,
            tc.tile_pool(name="ps5", bufs=4, space="PSUM") as ps5,
        ):
            # ---- x8 own-half + Q weights first (PE's first dependency) ----
            x8t = []
            wqt = []
            for j in range(4):
                t_ = x8p.tile([P, 2, T], F8, tag="x8", name=f"x8_{j}")
                nc.sync.dma_start(
                    t_[:, :, 0:OT],
                    x8[j * P : (j + 1) * P, :].rearrange("p (a t) -> p a t", a=2)[:, :, 0:OT],
                )
                x8t.append(t_)
                w_ = wp.tile([P, 2, D], F8, tag="w", name=f"wq{j}")
                nc.sync.dma_start(
                    w_[:],
                    wq8[j * P : (j + 1) * P, :].rearrange("p (a d) -> p a d", a=2),
                )
                wqt.append(w_)

            # biases packed [128, 26]: eb 0:8, qb 8:16, kb 16:24, wtb 24:26
            b4 = const.tile([P, 26], F32)
            nc.sync.dma_start(b4[:], bias4[:])
            ebt, qbt, kbt, wtbt = b4[:, 0:8], b4[:, 8:16], b4[:, 16:24], b4[:, 24:26]


            # ---- Q: own t cols (x8 cols 0:1024), paired over D for logits ----
            qt = [qp.tile([P, 2, OT], F8, tag="q", name=f"q{j}") for j in range(4)]
            for d in range(8):
                pt = ps1.tile([P, 1024], F32, tag="p1", name=f"psq{d}")
                for i in range(2):
                    for j in range(4):
                        nc.tensor.matmul(
                            pt[:, i * 512 : (i + 1) * 512],
                            wqt[j][:, :, d * P : (d + 1) * P],
                            x8t[j][:, :, i * 512 : (i + 1) * 512],
                            start=(j == 0), stop=(j == 3), perf_mode=DR,
                        )
                nc.scalar.activation(
                    qt[d // 2][:, d % 2, :], pt[:], Ident,
                    bias=qbt[:, d : d + 1], scale=1.0 / WS,
                )

            # peer half of x8 + K weights
            for j in range(4):
                nc.sync.dma_start(
                    x8t[j][:, :, OT:T],
                    x8[j * P : (j + 1) * P, :].rearrange("p (a t) -> p a t", a=2)[:, :, OT:T],
                )
            wkt = []
            for j in range(4):
                w_ = wp.tile([P, 2, D], F8, tag="w", name=f"wk{j}")
                nc.sync.dma_start(
                    w_[:],
                    wk8[j * P : (j + 1) * P, :].rearrange("p (a d) -> p a d", a=2),
                )
                wkt.append(w_)

            # ---- K: all 2048 s cols, paired over D ----
            kt = [kp.tile([P, 2, T], F8, tag="k", name=f"k{j}") for j in range(4)]
            for d in range(8):
                for q2 in range(2):
                    pt = ps1.tile([P, 1024], F32, tag="p1", name=f"psk{d}_{q2}")
                    for i in range(2):
                        for j in range(4):
                            nc.tensor.matmul(
                                pt[:, i * 512 : (i + 1) * 512],
                                wkt[j][:, :, d * P : (d + 1) * P],
                                x8t[j][:, :, q2 * 1024 + i * 512 : q2 * 1024 + (i + 1) * 512],
                                start=(j == 0), stop=(j == 3), perf_mode=DR,
                            )
                    dst = kt[d // 2][:, d % 2, q2 * 1024 : (q2 + 1) * 1024]
                    if q2 == 0:
                        nc.scalar.activation(
                            dst, pt[:], Ident, bias=kbt[:, d : d + 1], scale=1.0 / WS
                        )
                    else:
                        nc.vector.tensor_scalar(
                            dst, pt[:], 1.0 / WS, kbt[:, d : d + 1],
                            op0=MUL, op1=ADD,
                        )

            # remaining input DMAs (consumed in later phases)
            mskt = const.tile([P, SB, 512], BF)
            nc.sync.dma_start(mskt[:], masks.rearrange("(l p) c -> p l c", p=P))
            identt = const.tile([P, P], BF)
            nc.sync.dma_start(identt[:], ident[:])
            wvt = []
            for j in range(4):
                w_ = wp.tile([P, 2, D], F8, tag="w", name=f"wv{j}")
                nc.sync.dma_start(
                    w_[:],
                    wv8[j * P : (j + 1) * P, :].rearrange("p (a d) -> p a d", a=2),
                )
                wvt.append(w_)
            vbt = const.tile([1, D], BF)
            nc.sync.dma_start(vbt[:], vb[:])
            onest = const.tile([1, P], BF)
            nc.sync.dma_start(onest[:], onesv[:])
            xlt = []
            for j in range(4):
                t_ = xlp.tile([P, 2, OT], F8, tag="xl", name=f"xl{j}")
                nc.sync.dma_start(
                    t_[:], xl8[j * P : (j + 1) * P, :].rearrange("p (a t) -> p a t", a=2)
                )
                xlt.append(t_)
            mwht = []
            mwlt = []
            for j in range(4):
                w_ = wp.tile([P, 2, D], F8, tag="w", name=f"mwh{j}")
                nc.sync.dma_start(
                    w_[:],
                    mwh8[j * P : (j + 1) * P, :].rearrange("p (a d) -> p a d", a=2),
                )
                mwht.append(w_)
                w2 = wp.tile([P, 2, D], F8, tag="w", name=f"mwl{j}")
                nc.sync.dma_start(
                    w2[:],
                    mwl8[j * P : (j + 1) * P, :].rearrange("p (a d) -> p a d", a=2),
                )
                mwlt.append(w2)

            # ---- logits + E + local stats, V halves interleaved ----
            mpack = sxp.tile([P, SB], F32)
            zpack = sxp.tile([P, SB], F32)
            et = [ep.tile([P, 2, OT], E5, tag="e", name=f"e{j}") for j in range(8)]
            mt = [mp.tile([P, OT], BF, tag="m", name=f"m{d}") for d in range(8)]
            vt = [vp.tile([P, 2, D], BF, tag="v", name=f"v{j}") for j in range(8)]

            def v_half(l, i, eng):
                pt = ps5.tile([P, 512], F32, tag="p5", name=f"psv{l}_{i}")
                for j in range(4):
                    nc.tensor.matmul(
                        pt[:],
                        x8t[j][:, :, l * P : (l + 1) * P],
                        wvt[j][:, :, i * 512 : (i + 1) * 512],
                        start=(j == 0), stop=False, perf_mode=DR,
                    )
                nc.tensor.matmul(
                    pt[:], onest[0:1, :], vbt[0:1, i * 512 : (i + 1) * 512],
                    start=False, stop=True,
                )
                dst = vt[l // 2][:, l % 2, i * 512 : (i + 1) * 512]
                if eng == 0:
                    nc.scalar.activation(dst, pt[:], Ident, scale=1.0 / WS)
                else:
                    nc.vector.tensor_scalar_mul(dst, pt[:], 1.0 / WS)

            # V block schedule: Vc pairs {0,1,4,5} = s-blocks 0-3 and 8-11 first
            VB1 = [0, 1, 2, 3, 8, 9, 10, 11]      # during logits l=0..7 (full)
            VB2 = [4, 5, 12, 13]                   # halves during l=8..15
            VB3 = [6, 7, 14, 15]                   # after the logits loop

            def logits_block(l):
                bnd = (l % 8) // 4      # boundary tile index == first computed
                pt = ps1.tile([P, 1024], F32, tag="p1", name=f"psl{l}")
                for i in range(bnd, 2):
                    for j in range(4):
                        nc.tensor.matmul(
                            pt[:, i * 512 : (i + 1) * 512],
                            kt[j][:, :, l * P : (l + 1) * P],
                            qt[j][:, :, i * 512 : (i + 1) * 512],
                            start=(j == 0), stop=(j == 3 and i != bnd),
                            perf_mode=DR,
                        )
                    if i == bnd:
                        nc.tensor.matmul(
                            pt[:, i * 512 : (i + 1) * 512],
                            identt[:], mskt[:, l, :],
                            start=False, stop=True,
                        )
                if l % 8 >= 4:
                    nc.vector.memset(et[l // 2][:, l % 2, 0:512], 0.0)
                sl = pt[:, bnd * 512 : 1024]
                mcol = mpack[:, l : l + 1]
                nc.vector.reduce_max(mcol, sl, axis=X)
                negM = sxp.tile([P, 1], F32, tag="ng", bufs=SB, name=f"ng{l}")
                nc.vector.tensor_scalar_mul(negM, mcol, -1.0)
                nc.scalar.activation(
                    et[l // 2][:, l % 2, bnd * 512 : 1024], sl, Exp,
                    bias=negM[:, 0:1], accum_out=zpack[:, l : l + 1],
                )

            IVB = [0, 1, 2, 3, 8, 9, 10, 11, 4, 5, 12, 13]
            for l in range(SB):
                logits_block(l)
                if l < 12:
                    vl = IVB[l]
                    v_half(vl, 0, 0)
                    v_half(vl, 1, 0)

            # ---- stats exchange (one tiny AllGather per pair) ----
            spack = sxp.tile([P, 32], F32)
            nc.vector.tensor_copy(spack[:, 0:16], mpack[:])
            nc.vector.tensor_copy(spack[:, 16:32], zpack[:])
            nc.sync.dma_start(stats_in[:], spack[:])
            nc.gpsimd.collective_compute(
                "AllGather", mybir.AluOpType.bypass,
                ins=[stats_in[:]], outs=[stats_out[:]], replica_groups=RG,
            )
            gtop = sxp.tile([P, 32], F32)
            gbot = sxp.tile([P, 32], F32)
            nc.sync.dma_start(gtop[:], stats_out[0:P, :])
            nc.sync.dma_start(gbot[:], stats_out[P : 2 * P, :])

            # leftover V blocks (fill the collective window together with m)
            for n, vl in enumerate([6, 7, 14, 15]):
                v_half(vl, 0, n % 2)
                v_half(vl, 1, 1 - n % 2)

            # ---- m: split-fp8 (xh@Wh + xl@Wh + xh@Wl), own t cols ----
            for d in range(8):
                for i in range(2):
                    pt = ps5.tile([P, 512], F32, tag="p5", name=f"psm{d}_{i}")
                    nmm = 0
                    for j in range(4):
                        for lhs, rhs in (
                            (mwht[j], x8t[j][:, :, i * 512 : (i + 1) * 512]),
                            (mwht[j], xlt[j][:, :, i * 512 : (i + 1) * 512]),
                            (mwlt[j], x8t[j][:, :, i * 512 : (i + 1) * 512]),
                        ):
                            nmm += 1
                            nc.tensor.matmul(
                                pt[:], lhs[:, :, d * P : (d + 1) * P], rhs,
                                start=(nmm == 1), stop=(nmm == 12), perf_mode=DR,
                            )
                    nc.scalar.activation(
                        mt[d][:, i * 512 : (i + 1) * 512], pt[:], Ident,
                        bias=ebt[:, d : d + 1], scale=1.0 / WS,
                    )

            # ---- combine stats -> f[s] = exp(M_loc - Mg) / (32 Zg) ----
            oth = sxp.tile([P, 32], F32)
            nc.vector.tensor_scalar_mul(oth[:], gtop[:], wtbt[:, 0:1])
            nc.vector.scalar_tensor_tensor(
                oth[:], gbot[:], wtbt[:, 1:2], oth[:], op0=MUL, op1=ADD
            )
            oM = sxp.tile([P, 16], F32)
            oZ = sxp.tile([P, 16], F32)
            nc.vector.tensor_copy(oM[:, 0:8], oth[:, 8:16])
            nc.vector.tensor_copy(oM[:, 8:16], oth[:, 0:8])
            nc.vector.tensor_copy(oZ[:, 0:8], oth[:, 24:32])
            nc.vector.tensor_copy(oZ[:, 8:16], oth[:, 16:24])
            mg = sxp.tile([P, 16], F32)
            nc.vector.tensor_max(mg[:], mpack[:], oM[:])
            dm = sxp.tile([P, 16], F32)
            nc.vector.tensor_sub(dm[:], mpack[:], mg[:])
            expm = sxp.tile([P, 16], F32)
            nc.scalar.activation(expm[:], dm[:], Exp)
            nc.vector.tensor_sub(dm[:], oM[:], mg[:])
            expo = sxp.tile([P, 16], F32)
            nc.scalar.activation(expo[:], dm[:], Exp)
            zg = sxp.tile([P, 16], F32)
            nc.vector.tensor_mul(zg[:], zpack[:], expm[:])
            nc.vector.tensor_mul(oZ[:], oZ[:], expo[:])
            nc.vector.tensor_add(zg[:], zg[:], oZ[:])
            fsc = sxp.tile([P, 16], F32)
            nc.vector.reciprocal(fsc[:], zg[:])
            nc.vector.tensor_mul(fsc[:], fsc[:], expm[:])
            nc.vector.tensor_scalar_mul(fsc[:], fsc[:], 1.0 / 32.0)

            # ---- Vc = f * V (e5m2); read tile i=0 only needs pairs {0,1,4,5} ----
            vct = [vcp.tile([P, 2, D], E5, tag="vc", name=f"vc{j}") for j in range(8)]

            def vc_pair(j2, eng):
                for a in range(2):
                    l = 2 * j2 + a
                    dst = vct[j2][:, a, :]
                    src = vt[j2][:, a, :]
                    if eng == 0:
                        nc.scalar.activation(dst, src, Ident, scale=fsc[:, l : l + 1])
                    elif eng == 1:
                        nc.vector.tensor_scalar_mul(dst, src, fsc[:, l : l + 1])
                    else:
                        nc.gpsimd.tensor_scalar_mul(dst, src, fsc[:, l : l + 1])

            def read_tile(i, prs):
                for d in range(8):
                    pt = ps5.tile([P, 512], F32, tag="p5", name=f"psr{i}_{d}")
                    for idx, j2 in enumerate(prs):
                        nc.tensor.matmul(
                            pt[:],
                            vct[j2][:, :, d * P : (d + 1) * P],
                            et[j2][:, :, i * 512 : (i + 1) * 512],
                            start=(idx == 0), stop=(idx == len(prs) - 1),
                            perf_mode=DR,
                        )
                    osb = osp.tile([P, 512], BF, tag="os", name=f"os{i}_{d}")
                    nc.vector.scalar_tensor_tensor(
                        osb[:], mt[d][:, i * 512 : (i + 1) * 512], 1.0, pt[:],
                        op0=MUL, op1=ADD,
                    )
                    nc.sync.dma_start(
                        out_ext[d * P : (d + 1) * P, i * 512 : (i + 1) * 512], osb[:]
                    )

            for n, j2 in enumerate((0, 1, 4, 5)):
                vc_pair(j2, n % 2)
            read_tile(0, [0, 1, 4, 5])
            for n, j2 in enumerate((2, 3, 6, 7)):
                vc_pair(j2, n % 2)
            read_tile(1, list(range(8)))

    nc.compile()
    return nc


def _prep_inputs(minibatch, emb_w, emb_b, key_w, key_b, query_w, query_b,
                 value_w, value_b):
    bf = ml_dtypes.bfloat16
    f8 = ml_dtypes.float8_e4m3
    ewT_f = np.ascontiguousarray(emb_w.T).astype(np.float32)
    W_eq = ewT_f @ query_w.T.astype(np.float32)
    W_ek = ewT_f @ key_w.T.astype(np.float32)
    W_ev = ewT_f @ value_w.T.astype(np.float32)
    b_eq = emb_b @ query_w.T + query_b
    b_ek = emb_b @ key_w.T + key_b
    b_ev = emb_b @ value_w.T + value_b

    def pack_w(W):
        # [HID, D] -> [512, 2D]: row 128j+p, col a*D+d  holds W[256j+128a+p, d]
        W4 = W.reshape(4, 2, P, D)
        return np.ascontiguousarray(
            W4.transpose(0, 2, 1, 3).reshape(512, 2 * D)
        ).astype(f8)

    def pack_bias(v):
        return np.ascontiguousarray(v.reshape(8, P).T).astype(np.float32)

    # 16x scale lifts the ~0.013-rms folded weights out of fp8-subnormal range
    mwh_f32 = (WS * ewT_f).astype(f8).astype(np.float32)
    shared = {
        "wq8": pack_w(WS * W_eq),
        "wk8": pack_w(WS * W_ek),
        "wv8": pack_w(WS * W_ev),
        "mwh8": pack_w(mwh_f32),
        "mwl8": pack_w(WS * ewT_f - mwh_f32),
        "vb": (WS * b_ev).astype(bf)[None, :],
        "onesv": np.ones((1, P), dtype=bf),
        "ident": np.eye(P, dtype=bf),
    }

    in_maps = []
    for c in range(8):
        b, h = c // 2, c % 2
        xbT = minibatch[b].astype(np.float32)          # [HID, T]
        own = np.concatenate(
            [np.arange(P * (2 * u + h), P * (2 * u + h) + P) for u in range(8)]
        )
        peer = np.concatenate(
            [np.arange(P * (2 * u + 1 - h), P * (2 * u + 1 - h) + P) for u in range(8)]
        )
        perm = np.concatenate([own, peer])             # col order [own | peer]
        xp = xbT[:, perm]                              # [HID, 2048] permuted

        def pack_x(xc, w):
            x4 = xc.reshape(4, 2, P, w)
            return np.ascontiguousarray(
                x4.transpose(0, 2, 1, 3).reshape(512, 2 * w)
            ).astype(f8)

        x8c = pack_x(xp, T)
        xh = xp[:, 0:OT].astype(f8).astype(np.float32)
        xl8c = pack_x(xp[:, 0:OT] - xh, OT)            # split-fp8 residual of x

        # masks: per s-block l (permuted order), boundary-tile content
        mk = np.zeros((SB * P, 512), dtype=np.float32)
        for l in range(SB):
            bnd = (l % 8) // 4
            base_pos = 4 * bnd
            if l < 8:
                phys = 2 * l + h                 # own-parity s block
            else:
                phys = 2 * (l - 8) + (1 - h)     # peer-parity s block
            srow = P * phys + np.arange(P)[:, None]
            for pos in range(base_pos, base_pos + 4):
                tcol = P * (2 * pos + h) + np.arange(P)[None, :]
                blk = (tcol < srow) * NEGM
                mk[l * P : (l + 1) * P, (pos - base_pos) * P : (pos - base_pos + 1) * P] = blk
        bias4 = np.zeros((P, 26), dtype=np.float32)
        bias4[:, 0:8] = pack_bias(emb_b.astype(np.float32))
        bias4[:, 8:16] = pack_bias(b_eq.astype(np.float32))
        bias4[:, 16:24] = pack_bias(b_ek.astype(np.float32))
        bias4[:, 24] = 1.0 if h == 1 else 0.0    # weight of gathered rank0 rows
        bias4[:, 25] = 1.0 if h == 0 else 0.0    # weight of gathered rank1 rows
        in_maps.append(dict(
            shared,
            x8=x8c,
            xl8=xl8c,
            masks=mk.astype(bf),
            bias4=bias4,
        ))
    return in_maps


def kernel(**inputs):
    global LAST_EXEC_NS
    inputs = {k: np.asarray(v) for k, v in inputs.items()}
    if "nc" not in _CACHE:
        _CACHE["nc"] = _build_nc()
    nc = _CACHE["nc"]
    in_maps = _prep_inputs(**inputs)
    kw = {}
    if PROFILE:
        kw["trace"] = True
    res = run_bass_kernel_spmd(nc, in_maps, core_ids=list(range(8)), **kw)
    LAST_EXEC_NS = getattr(res, "exec_time_ns", None)
    out = np.empty((B, D, T), dtype=np.float32)
    for c in range(8):
        b, h = c // 2, c % 2
        o = np.asarray(res.results[c]["out"]).astype(np.float32)  # [D, OT]
        own = np.concatenate(
            [np.arange(P * (2 * u + h), P * (2 * u + h) + P) for u in range(8)]
        )
        out[b][:, own] = o
    return out
